# revision 1
# baseline (speedup 1.0000x reference)
"""Trainium2 Bass kernel for nn_AttentionBlock (causal attention, column softmax).

Computation (reference):
    Q/K/V = X @ W + b  per batch b of X[4, 4096, 512]
    logits[t,s] = <q_t, k_s>, causal mask (s>t -> -inf),
    probs = softmax over t (per column s) / sqrt(512)
    out = X + probs @ V

Sharding: 8 cores = (batch b in 0..3) x (half h in 0..1). Within a batch the
32 key-blocks (128 rows each) are split between the two halves so that both
halves get one block of every "extent class" c (blocks 2c, 2c+1 share the
query window [256c, 4096)), giving an identical SPMD program on every core
with balanced causal work. Masks are data, not program structure.

Q and K projections are folded into one matrix: logits^T[s,t] = k_s.q_t =
x_s (Wk Wq^T) x_t^T + (x_t Wq).bk + (x_s Wk).bq + bk.bq. The host passes
W2 = Wk @ Wq^T; the device computes A^T = (Wq Wk^T) X_sel^T (the cost of one
K^T projection) with (Wq bk) added per partition, and the logits matmul
contracts A^T against X^T directly — no Q^T projection at all. The per-key
term c_s = (x_s Wk).bq + bk.bq is host-computed and added as the ACT bias of
the exp. Per key-block: exp via ACT (row-sums via accum_out), diagonal-block
mask via DVE multiply, reciprocal row-sums folded into V rows, then AV
matmuls accumulate the partial output per 128-query block in PSUM. Host adds
the two partials and the residual. All matmuls bf16 with fp32 accumulation.
"""
import sys
if "/opt/trn_rl_repo" not in sys.path:
    sys.path.insert(0, "/opt/trn_rl_repo")

import numpy as np
import ml_dtypes

import concourse.bass as bass  # noqa: F401  (bass must import before tile)
import concourse.tile as tile
from concourse import bacc, mybir
from concourse.bass_utils import run_bass_kernel_spmd

bf16 = ml_dtypes.bfloat16
AFT = mybir.ActivationFunctionType
ALU = mybir.AluOpType

B, T, D = 4, 4096, 512      # K = V = D = 512
P = 128                     # partitions
NSLOT = 16                  # key blocks per core
CH = 512                    # chunk width (free dim per matmul)
INV_SQRT_K = float(1.0 / np.sqrt(np.float32(D)))

# slot i == class c; window start 256c, extent 4096 - 256c columns.
# Chunks are 512 wide with a trailing 256-wide chunk for odd c.
_EXT = [T - 256 * i for i in range(NSLOT)]
_POFF = np.concatenate([[0], np.cumsum(_EXT)]).astype(int)  # offsets into pall
PTOT = int(_POFF[-1])  # 34816


def _chunks(i):
    """(t0, width) chunks of slot i's query window [256i, 4096)."""
    out = []
    t0 = 256 * i
    while t0 < T:
        w = min(CH, T - t0)
        out.append((t0, w))
        t0 += w
    return out


def _build_program(reps=1, scratch_out=False, null_prog=False):
    """scratch_out: write results to internal DRAM and expose a tiny external
    output — used only for device-time measurement (removes the 64MB/call
    host transfer). null_prog: same I/O signature, no work (overhead calib).
    """
    nc = bacc.Bacc("TRN2", target_bir_lowering=False, debug=False, num_devices=8)
    dbf, df32 = mybir.dt.bfloat16, mybir.dt.float32

    XT = nc.dram_tensor("XT", [D, T], dbf, kind="ExternalInput").ap()
    XST = nc.dram_tensor("XST", [D, 2048], dbf, kind="ExternalInput").ap()
    W2 = nc.dram_tensor("W2", [D, D], dbf, kind="ExternalInput").ap()   # Wk @ Wq^T
    WV = nc.dram_tensor("WV", [D, D], dbf, kind="ExternalInput").ap()
    QBK = nc.dram_tensor("QBK", [P, 4], df32, kind="ExternalInput").ap()  # Wq @ bk
    CS = nc.dram_tensor("CS", [P, NSLOT], df32, kind="ExternalInput").ap()  # per-key bias
    BV = nc.dram_tensor("BV", [P, D], dbf, kind="ExternalInput").ap()
    MASK = nc.dram_tensor("MASK", [P, CH], dbf, kind="ExternalInput").ap()
    if scratch_out or null_prog:
        OUT = nc.dram_tensor("OUTS", [T, D], dbf).ap()  # internal scratch
        OUT2 = nc.dram_tensor("OUT2", [P, 4], df32, kind="ExternalOutput").ap()
    else:
        OUT = nc.dram_tensor("OUT", [T, D], dbf, kind="ExternalOutput").ap()
        OUT2 = None

    if null_prog:
        with tile.TileContext(nc) as tc:
            with tc.tile_pool(name="nsb", bufs=1) as sb:
                t = sb.tile([P, 4], df32, tag="t")
                nc.sync.dma_start(t[:], QBK[:])
                nc.sync.dma_start(OUT2[:], t[:])
        nc.compile()
        return nc

    with tile.TileContext(nc) as tc:
        with tc.tile_pool(name="persist", bufs=1) as pp, \
             tc.tile_pool(name="small", bufs=2) as sp, \
             tc.tile_pool(name="lpsum", bufs=4, space="PSUM") as lp, \
             tc.tile_pool(name="cpsum", bufs=2, space="PSUM") as cp:

            xt = pp.tile([P, 4 * T], dbf, tag="xt")        # X^T: [d | t]
            at = pp.tile([P, 4 * 2048], dbf, tag="at")     # A^T: [d' | s_local]
            vsc = pp.tile([P, NSLOT * CH], dbf, tag="vsc")  # V rows (later scaled)
            pall = pp.tile([P, PTOT], dbf, tag="pall")     # exp(logits^T) all slots
            qbk = pp.tile([P, 4], df32, tag="qbk")
            cs = pp.tile([P, NSLOT], df32, tag="cs")
            bv = pp.tile([P, D], dbf, tag="bv")
            mask = pp.tile([P, CH], dbf, tag="mask")

            def one_rep(rep):
                with tc.tile_pool(name=f"aph{rep}", bufs=1) as ap_, \
                     tc.tile_pool(name=f"apsum{rep}", bufs=2, space="PSUM") as aps:
                    xst = ap_.tile([P, 4 * 2048], dbf, tag="xst")  # X_sel^T: [d | s]
                    w2 = ap_.tile([P, 4 * D], dbf, tag="w2")       # [d | d']
                    wv = ap_.tile([P, 4 * D], dbf, tag="wv")

                    # phase-A-critical first: xst + w2 land on the 8 DMA queues
                    # in parallel; the big xt (phase B only) goes last.
                    for db in range(4):
                        nc.sync.dma_start(xst[:, 2048 * db:2048 * (db + 1)], XST[P * db:P * (db + 1), :])
                        nc.sync.dma_start(w2[:, D * db:D * (db + 1)], W2[P * db:P * (db + 1), :])
                    for db in range(4):
                        nc.sync.dma_start(wv[:, D * db:D * (db + 1)], WV[P * db:P * (db + 1), :])
                    nc.sync.dma_start(qbk[:], QBK[:])
                    nc.sync.dma_start(cs[:], CS[:])
                    nc.sync.dma_start(bv[:], BV[:])
                    nc.sync.dma_start(mask[:], MASK[:])
                    for db in range(4):
                        nc.sync.dma_start(xt[:, T * db:T * (db + 1)], XT[P * db:P * (db + 1), :])

                    # A^T[d', s] = sum_d (Wq Wk^T)[d', d] X_sel^T[d, s]  (+ Wq@bk per partition)
                    for kb in range(4):
                        for sc in range(4):
                            ps = aps.tile([P, CH], df32, tag="aps")
                            for db in range(4):
                                nc.tensor.matmul(
                                    ps[:],
                                    w2[:, D * db + P * kb: D * db + P * kb + P],
                                    xst[:, 2048 * db + CH * sc: 2048 * db + CH * (sc + 1)],
                                    start=(db == 0), stop=(db == 3),
                                )
                            nc.scalar.activation(
                                at[:, 2048 * kb + CH * sc: 2048 * kb + CH * (sc + 1)], ps[:],
                                AFT.Identity, bias=qbk[:, kb:kb + 1],
                            )
                    # V[s, v] = sum_d X_sel[s, d] Wv[d, v]  (+ bv broadcast)
                    for i in range(NSLOT):
                        ps = aps.tile([P, CH], df32, tag="aps")
                        for db in range(4):
                            nc.tensor.matmul(
                                ps[:],
                                xst[:, 2048 * db + P * i: 2048 * db + P * i + P],
                                wv[:, D * db: D * (db + 1)],
                                start=(db == 0), stop=(db == 3),
                            )
                        nc.vector.tensor_add(vsc[:, CH * i:CH * (i + 1)], ps[:], bv[:])

                # Phase B: per slot logits -> exp -> row sums -> fold 1/denom into V
                for i in range(NSLOT):
                    chunks = _chunks(i)
                    w0 = 256 * i
                    off = int(_POFF[i])
                    sums = sp.tile([P, 8], df32, tag="sums")
                    for e, (t0, w) in enumerate(chunks):
                        ps = lp.tile([P, CH], df32, tag="lg")
                        for kb in range(4):
                            nc.tensor.matmul(
                                ps[:, 0:w],
                                at[:, 2048 * kb + P * i: 2048 * kb + P * i + P],
                                xt[:, T * kb + t0: T * kb + t0 + w],
                                start=(kb == 0), stop=(kb == 3),
                            )
                        po = off + (t0 - w0)
                        if e == 0:
                            ptmp = sp.tile([P, CH], mybir.dt.bfloat16, tag="ptmp")
                            nc.scalar.activation(ptmp[:, 0:w], ps[:, 0:w], AFT.Exp,
                                                 bias=cs[:, i:i + 1])
                            nc.vector.tensor_mul(
                                pall[:, po:po + w], ptmp[:, 0:w], mask[:, 0:w]
                            )
                            nc.vector.tensor_reduce(
                                sums[:, 0:1], pall[:, po:po + w],
                                axis=mybir.AxisListType.X, op=ALU.add,
                            )
                        else:
                            nc.scalar.activation(
                                pall[:, po:po + w], ps[:, 0:w],
                                AFT.Exp, bias=cs[:, i:i + 1], accum_out=sums[:, e:e + 1],
                            )
                    den = sp.tile([P, 1], df32, tag="den")
                    nc.vector.tensor_reduce(den[:], sums[:, 0:len(chunks)],
                                            axis=mybir.AxisListType.X, op=ALU.add)
                    r2 = sp.tile([P, 1], df32, tag="r2")
                    nc.vector.reciprocal(r2[:], den[:])
                    nc.vector.tensor_scalar(
                        out=vsc[:, CH * i:CH * (i + 1)], in0=vsc[:, CH * i:CH * (i + 1)],
                        scalar1=r2[:], scalar2=INV_SQRT_K,
                        op0=ALU.mult, op1=ALU.mult,
                    )

                    # Phase C interleaved: t-blocks 2i, 2i+1 need only slots 0..i,
                    # so their AV matmuls can fill PE gaps during later softmaxes.
                    for tau in (2 * i, 2 * i + 1):
                        n = i + 1  # slots with window start <= 128*tau
                        ps = cp.tile([P, CH], df32, tag="avp")
                        for j in range(n):
                            tloc = tau - 2 * j
                            po = int(_POFF[j]) + P * tloc
                            nc.tensor.matmul(
                                ps[:], pall[:, po:po + P], vsc[:, CH * j:CH * (j + 1)],
                                start=(j == 0), stop=(j == n - 1),
                            )
                        st = sp.tile([P, CH], dbf, tag="st")
                        nc.vector.tensor_copy(st[:], ps[:])
                        nc.sync.dma_start(OUT[P * tau:P * (tau + 1), :], st[:])

            for rep in range(reps):
                one_rep(rep)

            if OUT2 is not None:
                fin = sp.tile([P, 4], df32, tag="fin")
                nc.gpsimd.memset(fin[:], 0.0)
                nc.sync.dma_start(OUT2[:], fin[:])

    nc.compile()
    return nc


_PROGRAM = None


def _get_program():
    global _PROGRAM
    if _PROGRAM is None:
        _PROGRAM = _build_program()
    return _PROGRAM


def _core_inputs(X, W2_b, Wv_b, QBK_h, BV_b, masks, wkbq, bkbq, b, h):
    """Per-core input map for core (b, h)."""
    Xb = X[b]
    XTb = np.ascontiguousarray(Xb.T).astype(bf16)
    sel = Xb.reshape(16, 2, P, D)[:, h].reshape(2048, D)
    XSTb = np.ascontiguousarray(sel.T).astype(bf16)
    # per-key-row logit bias c_s = (x_s Wk).bq + bk.bq, [2048] -> [128, 16]
    cvec = sel.astype(np.float64) @ wkbq + bkbq
    CS_h = np.ascontiguousarray(cvec.reshape(NSLOT, P).T).astype(np.float32)
    return {
        "XT": XTb, "XST": XSTb,
        "W2": W2_b, "WV": Wv_b,
        "QBK": QBK_h, "CS": CS_h, "BV": BV_b,
        "MASK": masks[h],
    }


def _prep_shared(Wk, bk, Wq, bq, Wv, bv):
    Wk64 = np.asarray(Wk, np.float64)
    Wq64 = np.asarray(Wq, np.float64)
    W2_b = np.ascontiguousarray(Wk64 @ Wq64.T).astype(bf16)       # lhsT for A^T
    Wv_b = np.ascontiguousarray(np.asarray(Wv)).astype(bf16)
    qbk = Wq64 @ np.asarray(bk, np.float64)                       # [512]
    QBK_h = np.ascontiguousarray(qbk.reshape(4, P).T).astype(np.float32)
    wkbq = Wk64 @ np.asarray(bq, np.float64)                      # [512]
    bkbq = float(np.asarray(bk, np.float64) @ np.asarray(bq, np.float64))
    BV_b = np.tile(np.asarray(bv).astype(bf16)[None, :], (P, 1))
    masks = np.zeros((2, P, CH), dtype=bf16)  # [h]
    s_loc = np.arange(P)[:, None]
    t_loc = np.arange(CH)[None, :]
    for h in range(2):
        masks[h] = (t_loc >= P * h + s_loc).astype(bf16)
    return W2_b, Wv_b, QBK_h, BV_b, masks, wkbq, bkbq


def kernel(minibatch, Wk, bk, Wq, bq, Wv, bv):
    X = np.asarray(minibatch, dtype=np.float32)
    nc = _get_program()
    shared = _prep_shared(Wk, bk, Wq, bq, Wv, bv)
    in_maps = [
        _core_inputs(X, *shared, b, h)
        for b in range(B) for h in range(2)
    ]
    last_exc = None
    for attempt in range(4):
        try:
            res = run_bass_kernel_spmd(nc, in_maps, list(range(2 * B)))
        except Exception as exc:  # transient device wedge — retry
            last_exc = exc
            continue
        out = X.copy()
        for b in range(B):
            out[b] += res.results[2 * b]["OUT"].astype(np.float32)
            out[b] += res.results[2 * b + 1]["OUT"].astype(np.float32)
        # transient device faults can surface as NaN/garbage — retry
        if not np.isnan(out).any() and np.abs(out).max() < 1e4:
            return out
    if last_exc is not None:
        raise last_exc
    return out

